# revision 4
# baseline (speedup 1.0000x reference)
"""GAT classifier (2-layer GAT + mean-pool + MLP) on 8 Trainium2 cores.

Strategy (edge parallelism per the sharding hint):
- Edges are sorted by dst and partitioned into 8 contiguous dst-slabs
  (6250 nodes per core); all edges into a node live on that node's core, so
  segment softmax and aggregation are core-local.
- Per core, edges are grouped into 128-node dst blocks. Within a block, edges
  are split into lo (src < 32768) / hi groups because dma_gather indices are
  int16; each group is padded to whole 128-edge tiles. Block tile counts are
  maxed across cores so all 8 cores run one SPMD program.
- Layer tables (h rows, bf16) live in DRAM; per-edge source features are
  fetched with dma_gather (256B rows L1, 512B rows [h|a_src] L2). Layer-1
  logits are streamed in precomputed on the host (a linear map of the input
  x). Layer-2 per-edge a_dst comes from a second narrow dma_gather over a
  core-local table indexed by local dst.
- Per 128-edge tile: ex = exp(leaky_relu(s)); messages ex*h are aggregated
  with a matmul against a 0/1 dst-selection matrix (built by one is_equal),
  accumulating [128 nodes, 128 feats + 4 denom] in PSUM per block. The
  softmax division happens once per node after aggregation.
- x1 slabs are AllGathered as the layer-2 table. Mean-pool partials are
  AllReduced; the tiny MLP is replicated.
- alpha1 = ex1/(denom1+1e-16) is assembled on the host from the exported
  per-edge ex and per-node denom.
"""
import sys

sys.path.insert(0, "/opt/trn_rl_repo")
import numpy as np
import ml_dtypes

import concourse.bacc as bacc
import concourse.bass as bass
import concourse.mybir as mybir
import concourse.tile as tile
from concourse import bass_utils
from concourse.library_config import mlp as mlp_lib

NCORES = 8
P = 128
H, C = 4, 32
HC = H * C
LO = 32768          # int16 gather index limit
GCHUNK = 32         # tiles per dma_gather call
NEG_SLOPE = 0.2

_CACHE = {}


def _pad128(n):
    return (n + P - 1) // P * P


def _wrap_idx(idx_flat):
    """int16 index stream -> [128, n/16] layout (j at [j%16, j//16], x8)."""
    n = len(idx_flat)
    assert n % 16 == 0
    arr = idx_flat.reshape(-1, 16).T.astype(np.int16)
    return np.tile(arr, (8, 1)).copy()


def _host_prep(x, edge_attr, edge_index, batch, params):
    """Sort/partition edges, build per-core streams and constants."""
    N, F = x.shape
    E = edge_index.shape[1]
    assert N % NCORES == 0
    SLAB = N // NCORES
    NBLK = (SLAB + P - 1) // P
    src = edge_index[0].astype(np.int64)
    dst = edge_index[1].astype(np.int64)

    (W1, att_src1, att_dst1, We1, att_edge1,
     W2, att_src2, att_dst2, We2, att_edge2) = params

    # A matrices: [HC, H] block-diagonal per head
    def amat(att):
        A = np.zeros((HC, H), np.float32)
        for h in range(H):
            A[h * C:(h + 1) * C, h] = att[h]
        return A

    A_src1, A_dst1 = amat(att_src1), amat(att_dst1)
    A_src2, A_dst2 = amat(att_src2), amat(att_dst2)
    # a_edge[e, h] = edge_attr[e,0] * (We[0, h*C:(h+1)*C] @ att_edge[h])
    k1 = np.array([We1[0, h * C:(h + 1) * C] @ att_edge1[h] for h in range(H)], np.float32)
    k2 = np.array([We2[0, h * C:(h + 1) * C] @ att_edge2[h] for h in range(H)], np.float32)
    ea = edge_attr[:, 0].astype(np.float32)
    aedge1 = ea[:, None] * k1[None, :]
    aedge2 = ea[:, None] * k2[None, :]
    # layer-1 per-node logit pieces (host: linear in input x)
    asrc1_n = (x @ W1 @ A_src1).astype(np.float32)
    adst1_n = (x @ W1 @ A_dst1).astype(np.float32)
    s1_edge = asrc1_n[src] + adst1_n[dst] + aedge1      # [E, 4]

    core_of = dst // SLAB
    ldst = dst - core_of * SLAB
    blk = ldst // P
    dst_in_blk = ldst % P
    is_lo = src < LO

    # per (core, block, half) edge lists
    lists = [[[[], []] for _ in range(NBLK)] for _ in range(NCORES)]
    order = np.lexsort((src, blk, core_of))  # group by core, block (stable)
    for e in order:
        lists[core_of[e]][blk[e]][0 if is_lo[e] else 1].append(e)

    KLo = [max(1, (max(len(lists[c][b][0]) for c in range(NCORES)) + P - 1) // P)
           for b in range(NBLK)]
    KHi = [max(1, (max(len(lists[c][b][1]) for c in range(NCORES)) + P - 1) // P)
           for b in range(NBLK)]
    T_lo, T_hi = sum(KLo), sum(KHi)
    T = T_lo + T_hi
    SL, SH = T_lo * P, T_hi * P

    per_core = []
    for c in range(NCORES):
        idx_lo = np.zeros(SL, np.int64)
        idx_hi = np.zeros(SH, np.int64)
        dstcol = np.full(T * P, -1.0, np.float32)      # slot -> dst_in_blk, pad -1
        dsti = np.zeros(T * P, np.int64)               # slot -> local dst (pads 0)
        s1p = np.zeros((T * P, H), np.float32)
        ae2 = np.zeros((T * P, H), np.float32)
        orig = np.full(T * P, -1, np.int64)            # slot -> original edge id
        lo_pos, hi_pos, slot = 0, 0, 0
        for b in range(NBLK):
            for half, K, posref in ((0, KLo[b], "lo"), (1, KHi[b], "hi")):
                es = lists[c][b][half]
                nslots = K * P
                for j, e in enumerate(es):
                    sl = slot + j
                    if half == 0:
                        idx_lo[lo_pos + j] = src[e]
                    else:
                        idx_hi[hi_pos + j] = src[e] - LO
                    dstcol[sl] = float(dst_in_blk[e])
                    dsti[sl] = ldst[e]
                    s1p[sl] = s1_edge[e]
                    ae2[sl] = aedge2[e]
                    orig[sl] = e
                if half == 0:
                    lo_pos += nslots
                else:
                    hi_pos += nslots
                slot += nslots
        pc = {
            "idx_lo": _wrap_idx(idx_lo.astype(np.int16)),
            "idx_hi": _wrap_idx(idx_hi.astype(np.int16)),
            "idx_dst": _wrap_idx(dsti.astype(np.int16)),
            # [128, T] / [128, 4T] layouts: slot s=(t,p) at [p, t...]
            "dstcol": dstcol.reshape(T, P).T.copy(),
            "s1p": s1p.reshape(T, P, H).transpose(1, 0, 2).reshape(P, T * H).copy(),
            "ae2": ae2.reshape(T, P, H).transpose(1, 0, 2).reshape(P, T * H).copy(),
            "orig": orig,
            "dsti": dsti,
            "x_slab": np.ascontiguousarray(x[c * SLAB:(c + 1) * SLAB]).astype(np.float32),
        }
        per_core.append(pc)

    meta = dict(N=N, F=F, E=E, SLAB=SLAB, NBLK=NBLK, KLo=KLo, KHi=KHi,
                T=T, T_lo=T_lo, T_hi=T_hi, SL=SL, SH=SH,
                A_src2=A_src2, A_dst2=A_dst2)
    return per_core, meta


def _build_module(meta, consts):
    N, F, SLAB, NBLK = meta["N"], meta["F"], meta["SLAB"], meta["NBLK"]
    KLo, KHi, T, T_lo, T_hi = meta["KLo"], meta["KHi"], meta["T"], meta["T_lo"], meta["T_hi"]
    SL, SH = meta["SL"], meta["SH"]
    SLAB_PAD = _pad128(SLAB)
    f32, bf16, i16 = mybir.dt.float32, mybir.dt.bfloat16, mybir.dt.int16
    AF = mybir.ActivationFunctionType
    OP = mybir.AluOpType

    nc = bacc.Bacc("TRN2", target_bir_lowering=False)
    # per-core inputs
    x_in = nc.dram_tensor("x_slab", [SLAB, F], f32, kind="ExternalInput")
    ilo_in = nc.dram_tensor("idx_lo", [P, SL // 16], i16, kind="ExternalInput")
    ihi_in = nc.dram_tensor("idx_hi", [P, SH // 16], i16, kind="ExternalInput")
    idst_in = nc.dram_tensor("idx_dst", [P, T * P // 16], i16, kind="ExternalInput")
    dstc_in = nc.dram_tensor("dstcol", [P, T], f32, kind="ExternalInput")
    s1p_in = nc.dram_tensor("s1p", [P, T * H], f32, kind="ExternalInput")
    ae2_in = nc.dram_tensor("ae2", [P, T * H], f32, kind="ExternalInput")
    gid_in = nc.dram_tensor("gidcol", [P, NBLK], f32, kind="ExternalInput")
    # shared consts
    cW1 = nc.dram_tensor("W1", [F, HC], f32, kind="ExternalInput")
    cW2 = nc.dram_tensor("W2", [HC, HC], f32, kind="ExternalInput")
    cA2 = nc.dram_tensor("A2", [HC, 2 * H], f32, kind="ExternalInput")
    cIottt = nc.dram_tensor("iota_row", [P, P], f32, kind="ExternalInput")
    cIotg = nc.dram_tensor("iota_g", [P, 64], f32, kind="ExternalInput")
    cIdent = nc.dram_tensor("ident", [P, P], f32, kind="ExternalInput")
    cB1 = nc.dram_tensor("b1_bcast", [P, HC], f32, kind="ExternalInput")
    cB2 = nc.dram_tensor("b2_bcast", [P, HC], f32, kind="ExternalInput")
    cW1m = nc.dram_tensor("W1m", [HC, 64], f32, kind="ExternalInput")
    cb1m = nc.dram_tensor("b1m", [1, 64], f32, kind="ExternalInput")
    cW2m = nc.dram_tensor("W2m", [64, 4], f32, kind="ExternalInput")
    cb2m = nc.dram_tensor("b2m", [1, 4], f32, kind="ExternalInput")
    cOnes = nc.dram_tensor("ones1", [1, 64], f32, kind="ExternalInput")
    # outputs
    logit_out = nc.dram_tensor("logits", [64, 4], f32, kind="ExternalOutput")
    ex1_out = nc.dram_tensor("ex1", [P, T * H], f32, kind="ExternalOutput")
    den1_out = nc.dram_tensor("den1", [P, NBLK * H], f32, kind="ExternalOutput")
    # internal DRAM
    adst2_dram = nc.dram_tensor("adst2t", [SLAB_PAD, 64], f32)

    with tile.TileContext(nc) as tc:
        with (
            tc.tile_pool(name="cst", bufs=1) as cst,
            tc.tile_pool(name="big", bufs=1) as big,
            tc.tile_pool(name="sb", bufs=3) as sb,
            tc.tile_pool(name="ring", bufs=2) as ring,
            tc.tile_pool(name="ps", bufs=1, space="PSUM") as ps,
            tc.tile_pool(name="ps1", bufs=2, space="PSUM") as ps1,
            tc.tile_pool(name="dram", bufs=1, space="DRAM") as dram,
        ):
            nc.gpsimd.load_library(mlp_lib)

            def sl2(buf, col, a=None, b=None):
                ap = buf[:, col:col + 1, :] if a is None else buf[:, col:col + 1, a:b]
                return ap.squeeze(1)

            def load_const(t_in, shape, dtype=f32, tag=None):
                t = cst.tile(shape, dtype, tag=tag or t_in.name)
                nc.sync.dma_start(t[:], t_in[:])
                return t

            W1_t = load_const(cW1, [F, HC])
            W2_t = load_const(cW2, [HC, HC])
            A2_t = load_const(cA2, [HC, 2 * H])
            iota_t = load_const(cIottt, [P, P])
            iotg_t = load_const(cIotg, [P, 64])
            id_t = load_const(cIdent, [P, P])
            b1b_t = load_const(cB1, [P, HC])
            b2b_t = load_const(cB2, [P, HC])
            W1m_t = load_const(cW1m, [HC, 64])
            b1m_t = load_const(cb1m, [1, 64])
            W2m_t = load_const(cW2m, [64, 4])
            b2m_t = load_const(cb2m, [1, 4])
            ones_t = load_const(cOnes, [1, 64])
            gid_t = load_const(gid_in, [P, NBLK])
            dstc_t = load_const(dstc_in, [P, T])
            s1p_t = big.tile([P, T * H], f32, tag="s1p")
            nc.sync.dma_start(s1p_t[:], s1p_in[:])
            ae2_t = big.tile([P, T * H], f32, tag="ae2")
            nc.sync.dma_start(ae2_t[:], ae2_in[:])
            ilo_t = big.tile([P, SL // 16], i16, tag="ilo")
            nc.sync.dma_start(ilo_t[:], ilo_in[:])
            ihi_t = big.tile([P, SH // 16], i16, tag="ihi")
            nc.sync.dma_start(ihi_t[:], ihi_in[:])
            idst_t = big.tile([P, T * P // 16], i16, tag="idst")
            nc.sync.dma_start(idst_t[:], idst_in[:])

            ex1_st = big.tile([P, T * H], f32, tag="ex1st")
            den_st = big.tile([P, NBLK * H], f32, tag="denst")
            x1_slab = big.tile([P, NBLK, HC], f32, tag="x1slab")
            x2pool = big.tile([P, NBLK, 132], bf16, tag="x2pool")
            nc.vector.memset(x2pool[:], 0.0)
            nc.vector.memset(x2pool[:, :, 128:129], 1.0)

            # collective buffers
            ag1_in = dram.tile([SLAB, HC], bf16, tag="ag1in")
            table1 = dram.tile([N, HC], bf16, tag="table1")
            ag2_in = dram.tile([SLAB, 256], bf16, tag="ag2in")
            table2 = dram.tile([N, 256], bf16, tag="table2")
            ar_in = dram.tile([64, 132], f32, tag="arin")
            ar_out = dram.tile([64, 132], f32, tag="arout")

            # ---------- phase 0: layer-1 table ----------
            for t in range(NBLK):
                rows = min(P, SLAB - t * P)
                x_t = sb.tile([P, F], f32, tag="ph0x")
                nc.sync.dma_start(x_t[:rows, :], x_in[t * P:t * P + rows, :])
                xT_p = ps.tile([P, P], f32, tag="ph0T")
                nc.tensor.transpose(xT_p[:], x_t[:], id_t[:])
                xT_s = sb.tile([P, P], f32, tag="ph0Ts")
                nc.vector.tensor_copy(xT_s[:], xT_p[:])
                h1_p = ps.tile([P, HC], f32, tag="ph0h")
                nc.tensor.matmul(h1_p[:], xT_s[:], W1_t[:], start=True, stop=True)
                h1_b = sb.tile([P, HC], bf16, tag="ph0hb")
                nc.vector.tensor_copy(h1_b[:], h1_p[:])
                nc.sync.dma_start(ag1_in[t * P:t * P + rows, :], h1_b[:rows, :])
            nc.gpsimd.collective_compute(
                "AllGather", OP.bypass, replica_groups=[list(range(NCORES))],
                ins=[ag1_in.opt()], outs=[table1.opt()])

            # ---------- edge phase (shared for both layers) ----------
            def edge_phase(layer):
                tbl = table1 if layer == 1 else table2
                elem = HC if layer == 1 else 256
                # gather chunk schedules for lo/hi streams
                chunks = {}
                for half, Ttot, it, base in (
                    ("lo", T_lo, ilo_t, 0), ("hi", T_hi, ihi_t, LO)):
                    lst = []
                    for c0 in range(0, Ttot, GCHUNK):
                        nt = min(GCHUNK, Ttot - c0)
                        lst.append((c0, nt))
                    chunks[half] = lst
                g_bufs = {}

                def get_g(half, st_tile):
                    # returns (buf, col) for stream-tile index
                    cidx = st_tile // GCHUNK
                    c0 = cidx * GCHUNK
                    nt = min(GCHUNK, (T_lo if half == "lo" else T_hi) - c0)
                    key = (half, cidx)
                    if key not in g_bufs:
                        it = ilo_t if half == "lo" else ihi_t
                        src_ap = tbl[:, :] if (half == "lo" or N <= LO) else tbl[LO:, :]
                        g = ring.tile([P, GCHUNK, elem], bf16, tag=f"g{layer}")
                        nc.gpsimd.dma_gather(
                            g[:, :nt, :], src_ap, it[:, c0 * 8:(c0 + nt) * 8],
                            nt * P, nt * P, elem, single_packet=False)
                        gd = None
                        g_bufs[key] = g
                    return g_bufs[key], st_tile - c0

                gd_bufs = {}

                def get_gd(gt):
                    cidx = gt // GCHUNK
                    c0 = cidx * GCHUNK
                    nt = min(GCHUNK, T - c0)
                    if cidx not in gd_bufs:
                        g = ring.tile([P, GCHUNK, 64], f32, tag="gd")
                        nc.gpsimd.dma_gather(
                            g[:, :nt, :], adst2_dram[:, :],
                            idst_t[:, c0 * 8:(c0 + nt) * 8],
                            nt * P, nt * P, 64, single_packet=False)
                        gd_bufs[cidx] = g
                    return gd_bufs[cidx], gt - c0

                lo_t, hi_t, gt = 0, 0, 0
                for b in range(NBLK):
                    out_p = ps1.tile([P, 132], f32, tag="out")
                    ntile = KLo[b] + KHi[b]
                    done = 0
                    for half, K in (("lo", KLo[b]), ("hi", KHi[b])):
                        for k in range(K):
                            st = lo_t if half == "lo" else hi_t
                            gbuf, gcol = get_g(half, st)
                            # s logits [P, 4]
                            if layer == 1:
                                s_ap = s1p_t[:, gt * H:(gt + 1) * H]
                            else:
                                gdbuf, gdcol = get_gd(gt)
                                t0 = sb.tile([P, H], f32, tag="t0")
                                nc.vector.tensor_tensor(
                                    t0[:], sl2(gbuf, gcol, 128, 132),
                                    ae2_t[:, gt * H:(gt + 1) * H], OP.add)
                                s_t2 = sb.tile([P, H], f32, tag="s2")
                                nc.vector.tensor_tensor(
                                    s_t2[:], t0[:], sl2(gdbuf, gdcol, 0, H), OP.add)
                                s_ap = s_t2[:]
                            lk1 = sb.tile([P, H], f32, tag="lk1")
                            nc.vector.tensor_scalar_mul(lk1[:], s_ap, NEG_SLOPE)
                            lk = sb.tile([P, H], f32, tag="lk")
                            nc.vector.tensor_tensor(lk[:], lk1[:], s_ap, OP.max)
                            msgx = sb.tile([P, 132], bf16, tag="msgx")
                            nc.scalar.activation(msgx[:, 128:132], lk[:], AF.Exp)
                            if layer == 1:
                                nc.scalar.activation(
                                    ex1_st[:, gt * H:(gt + 1) * H], lk[:], AF.Exp)
                            exv = msgx[:, 128:132].unsqueeze(2).to_broadcast([P, H, C])
                            nc.vector.tensor_tensor(
                                msgx[:, :128].rearrange("p (h c) -> p h c", h=H),
                                sl2(gbuf, gcol, 0, 128).rearrange("p (h c) -> p h c", h=H),
                                exv, OP.mult)
                            S_t = sb.tile([P, P], bf16, tag="S")
                            nc.vector.tensor_scalar(
                                S_t[:], iota_t[:], dstc_t[:, gt:gt + 1], None, OP.is_equal)
                            nc.tensor.matmul(out_p[:], S_t[:], msgx[:],
                                             start=(done == 0), stop=(done == ntile - 1))
                            done += 1
                            gt += 1
                            if half == "lo":
                                lo_t += 1
                            else:
                                hi_t += 1
                    # block epilogue
                    if layer == 1:
                        nc.vector.tensor_copy(den_st[:, b * H:(b + 1) * H], out_p[:, 128:132])
                    dpe = sb.tile([P, H], f32, tag="dpe")
                    nc.vector.tensor_scalar_add(dpe[:], out_p[:, 128:132], 1e-16)
                    drec = sb.tile([P, H], f32, tag="drec")
                    nc.vector.reciprocal(drec[:], dpe[:])
                    xo = sb.tile([P, HC], f32, tag="xo")
                    nc.vector.tensor_tensor(
                        xo[:].rearrange("p (h c) -> p h c", h=H),
                        out_p[:, :128].rearrange("p (h c) -> p h c", h=H),
                        drec[:].unsqueeze(2).to_broadcast([P, H, C]), OP.mult)
                    bias_t = b1b_t if layer == 1 else b2b_t
                    nc.vector.tensor_tensor(xo[:], xo[:], bias_t[:], OP.add)
                    # elu(xo) = max(xo,0) + exp(min(xo,0)) - 1
                    mneg = sb.tile([P, HC], f32, tag="mneg")
                    nc.vector.tensor_scalar_min(mneg[:], xo[:], 0.0)
                    epos = sb.tile([P, HC], f32, tag="epos")
                    nc.scalar.activation(epos[:], mneg[:], AF.Exp)
                    rpos = sb.tile([P, HC], f32, tag="rpos")
                    nc.vector.tensor_scalar_max(rpos[:], xo[:], 0.0)
                    xn = sb.tile([P, HC], f32, tag="xn")
                    nc.vector.tensor_tensor(xn[:], epos[:], rpos[:], OP.add)
                    if layer == 1:
                        nc.vector.tensor_scalar_add(sl2(x1_slab, b), xn[:], -1.0)
                    else:
                        nc.vector.tensor_scalar_add(sl2(x2pool, b, 0, 128), xn[:], -1.0)

            edge_phase(1)

            # ---------- phase 0': layer-2 table ----------
            adst_st = sb.tile([P, NBLK, H], f32, tag="adstst")
            for t in range(NBLK):
                rows = min(P, SLAB - t * P)
                xT_p = ps.tile([P, P], f32, tag="ph0T")
                nc.tensor.transpose(xT_p[:], sl2(x1_slab, t), id_t[:])
                xT_s = sb.tile([P, P], f32, tag="ph0Ts")
                nc.vector.tensor_copy(xT_s[:], xT_p[:])
                h2_p = ps.tile([P, HC], f32, tag="ph0h")
                nc.tensor.matmul(h2_p[:], xT_s[:], W2_t[:], start=True, stop=True)
                h2T_p = ps.tile([P, P], f32, tag="ph0hT")
                nc.tensor.matmul(h2T_p[:], W2_t[:], xT_s[:], start=True, stop=True)
                h2T_s = sb.tile([P, P], f32, tag="ph0hTs")
                nc.vector.tensor_copy(h2T_s[:], h2T_p[:])
                aa_p = ps.tile([P, 2 * H], f32, tag="ph0aa")
                nc.tensor.matmul(aa_p[:], h2T_s[:], A2_t[:], start=True, stop=True)
                row_t = sb.tile([P, 256], bf16, tag="ph0row")
                nc.vector.memset(row_t[:], 0.0)
                nc.vector.tensor_copy(row_t[:, :128], h2_p[:])
                nc.vector.tensor_copy(row_t[:, 128:132], aa_p[:, 0:H])
                nc.vector.tensor_copy(sl2(adst_st, t), aa_p[:, H:2 * H])
                nc.sync.dma_start(ag2_in[t * P:t * P + rows, :], row_t[:rows, :])
            adst_view = adst2_dram[:].rearrange("(t p) c -> p t c", p=P)
            nc.sync.dma_start(adst_view[:, :, 0:H], adst_st[:])
            nc.gpsimd.collective_compute(
                "AllGather", OP.bypass, replica_groups=[list(range(NCORES))],
                ins=[ag2_in.opt()], outs=[table2.opt()])

            edge_phase(2)

            # ---------- pooling ----------
            pool_p = ps1.tile([64, 132], f32, tag="pool")
            for b in range(NBLK):
                Sg = sb.tile([P, 64], bf16, tag="Sg")
                nc.vector.tensor_scalar(
                    Sg[:], iotg_t[:], gid_t[:, b:b + 1], None, OP.is_equal)
                nc.tensor.matmul(pool_p[:], Sg[:], sl2(x2pool, b),
                                 start=(b == 0), stop=(b == NBLK - 1))
            pool_s = sb.tile([64, 132], f32, tag="pools")
            nc.vector.tensor_copy(pool_s[:], pool_p[:])
            nc.gpsimd.dma_start(ar_in[:], pool_s[:])
            nc.gpsimd.collective_compute(
                "AllReduce", OP.add, replica_groups=[list(range(NCORES))],
                ins=[ar_in.opt()], outs=[ar_out.opt()])
            arr = sb.tile([64, 132], f32, tag="arr")
            nc.sync.dma_start(arr[:], ar_out[:])

            # ---------- MLP ----------
            cnt = sb.tile([64, 1], f32, tag="cnt")
            nc.vector.tensor_scalar_max(cnt[:], arr[:, 128:129], 1.0)
            crec = sb.tile([64, 1], f32, tag="crec")
            nc.vector.reciprocal(crec[:], cnt[:])
            gmat = sb.tile([64, 128], f32, tag="gmat")
            nc.vector.tensor_scalar(gmat[:], arr[:, :128], crec[:], None, OP.mult)
            gT_p = ps.tile([P, 64], f32, tag="ph0h")
            nc.tensor.transpose(gT_p[:], gmat[:], id_t[:64, :64])
            gT_s = sb.tile([P, 64], f32, tag="gTs")
            nc.vector.tensor_copy(gT_s[:], gT_p[:])
            m1_p = ps.tile([64, 64], f32, tag="ph0aa")
            nc.tensor.matmul(m1_p[:], gT_s[:], W1m_t[:], start=True, stop=False)
            nc.tensor.matmul(m1_p[:], ones_t[:], b1m_t[:], start=False, stop=True)
            hrelu = sb.tile([64, 64], f32, tag="hrelu")
            nc.vector.tensor_scalar_max(hrelu[:], m1_p[:], 0.0)
            hT_p = ps.tile([64, 64], f32, tag="ph0hT")
            nc.tensor.transpose(hT_p[:], hrelu[:], id_t[:64, :64])
            hT_s = sb.tile([64, 64], f32, tag="hTs")
            nc.vector.tensor_copy(hT_s[:], hT_p[:])
            m2_p = ps.tile([64, 4], f32, tag="ph0T")
            nc.tensor.matmul(m2_p[:], hT_s[:], W2m_t[:], start=True, stop=False)
            nc.tensor.matmul(m2_p[:], ones_t[:, :64], b2m_t[:], start=False, stop=True)
            lg = sb.tile([64, 4], f32, tag="lg")
            nc.vector.tensor_copy(lg[:], m2_p[:])
            nc.sync.dma_start(logit_out[:], lg[:])

            # stream outputs
            nc.sync.dma_start(ex1_out[:], ex1_st[:])
            nc.sync.dma_start(den1_out[:], den_st[:])
    nc.compile()
    return nc


def kernel(x, edge_attr, W1, att_src1, att_dst1, We1, att_edge1, b1,
           W2, att_src2, att_dst2, We2, att_edge2, b2,
           mlp_W1, mlp_b1, bn_gamma, bn_beta, bn_mean, bn_var, mlp_W2, mlp_b2,
           edge_index, batch):
    x = np.asarray(x, np.float32)
    edge_attr = np.asarray(edge_attr, np.float32)
    edge_index_in = edge_index
    edge_index = np.asarray(edge_index)
    batch = np.asarray(batch)
    params = tuple(np.asarray(p, np.float32) for p in (
        W1, att_src1, att_dst1, We1, att_edge1,
        W2, att_src2, att_dst2, We2, att_edge2))
    b1 = np.asarray(b1, np.float32)
    b2 = np.asarray(b2, np.float32)

    per_core, meta = _host_prep(x, edge_attr, edge_index, batch, params)
    N, E, SLAB, NBLK, T = meta["N"], meta["E"], meta["SLAB"], meta["NBLK"], meta["T"]

    # consts
    ident = np.eye(P, dtype=np.float32)
    iota_row = np.broadcast_to(np.arange(P, dtype=np.float32)[None, :], (P, P)).copy()
    iota_g = np.broadcast_to(np.arange(64, dtype=np.float32)[None, :], (P, 64)).copy()
    b1_bcast = np.broadcast_to(b1[None, :], (P, HC)).copy()
    b2_bcast = np.broadcast_to(b2[None, :], (P, HC)).copy()
    A2cat = np.concatenate([meta["A_src2"], meta["A_dst2"]], 1).astype(np.float32)
    # fold BN (eval) into mlp layer 1
    scale = (np.asarray(bn_gamma, np.float32)
             / np.sqrt(np.asarray(bn_var, np.float32) + 1e-5))
    shift = np.asarray(bn_beta, np.float32) - np.asarray(bn_mean, np.float32) * scale
    W1m = (np.asarray(mlp_W1, np.float32) * scale[None, :]).astype(np.float32)
    b1m = (np.asarray(mlp_b1, np.float32) * scale + shift).astype(np.float32)[None, :]
    W2m = np.asarray(mlp_W2, np.float32)
    b2m = np.asarray(mlp_b2, np.float32)[None, :]
    ones1 = np.ones((1, 64), np.float32)
    gid = np.asarray(batch, np.int64)

    shared = {
        "W1": params[0], "W2": params[5], "A2": A2cat,
        "iota_row": iota_row, "iota_g": iota_g, "ident": ident,
        "b1_bcast": b1_bcast, "b2_bcast": b2_bcast,
        "W1m": W1m, "b1m": b1m, "W2m": W2m, "b2m": b2m, "ones1": ones1,
    }
    key = (N, E, tuple(meta["KLo"]), tuple(meta["KHi"]))
    if key not in _CACHE:
        _CACHE[key] = _build_module(meta, shared)
    nc = _CACHE[key]

    in_maps = []
    for c in range(NCORES):
        pc = per_core[c]
        gidc = np.full((NBLK * P,), -1.0, np.float32)
        gslab = gid[c * SLAB:(c + 1) * SLAB].astype(np.float32)
        gidc[:SLAB] = gslab
        m = dict(shared)
        m.update({
            "x_slab": pc["x_slab"], "idx_lo": pc["idx_lo"], "idx_hi": pc["idx_hi"],
            "idx_dst": pc["idx_dst"], "dstcol": pc["dstcol"],
            "s1p": pc["s1p"], "ae2": pc["ae2"],
            "gidcol": gidc.reshape(NBLK, P).T.copy(),
        })
        in_maps.append(m)

    res = bass_utils.run_bass_kernel_spmd(nc, in_maps, core_ids=list(range(NCORES)))
    outs = res.results

    logits = outs[0]["logits"].astype(np.float32)

    # alpha1 on host: ex / (denom[dst] + 1e-16)
    alpha = np.zeros((E, H), np.float32)
    for c in range(NCORES):
        pc = per_core[c]
        ex = outs[c]["ex1"].reshape(P, T, H).transpose(1, 0, 2).reshape(T * P, H)
        den = outs[c]["den1"].reshape(P, NBLK, H).transpose(1, 0, 2).reshape(NBLK * P, H)
        real = pc["orig"] >= 0
        d_lidx = pc["dsti"][real]
        blkpos = (d_lidx // P) * P + (d_lidx % P)
        alpha[pc["orig"][real]] = ex[real] / (den[blkpos] + 1e-16)

    return logits, (edge_index_in, alpha)


# revision 6
# speedup vs baseline: 1.9283x; 1.9283x over previous
"""GAT classifier (2-layer GAT + mean-pool + MLP) on 8 Trainium2 cores.

Strategy (edge parallelism per the sharding hint):
- Edges are sorted by dst and partitioned into 8 contiguous dst-slabs
  (6250 nodes per core); all edges into a node live on that node's core, so
  segment softmax and aggregation are core-local.
- Per core, edges are grouped into 128-node dst blocks. Within a block, edges
  are split into lo (src < 32768) / hi groups because dma_gather indices are
  int16; each group is padded to whole 128-edge tiles. Block tile counts are
  maxed across cores so all 8 cores run one SPMD program.
- Layer tables (h rows, bf16) live in DRAM; per-edge source features are
  fetched with dma_gather (256B rows L1, 512B rows [h|a_src] L2). Layer-1
  logits are streamed in precomputed on the host (a linear map of the input
  x). Layer-2 per-edge a_dst comes from a second narrow dma_gather over a
  core-local table indexed by local dst.
- Per 128-edge tile: ex = exp(leaky_relu(s)); messages ex*h are aggregated
  with a matmul against a 0/1 dst-selection matrix (built by one is_equal),
  accumulating [128 nodes, 128 feats + 4 denom] in PSUM per block. The
  softmax division happens once per node after aggregation.
- x1 slabs are AllGathered as the layer-2 table. Mean-pool partials are
  AllReduced; the tiny MLP is replicated.
- alpha1 = ex1/(denom1+1e-16) is assembled on the host from the exported
  per-edge ex and per-node denom.
"""
import sys

sys.path.insert(0, "/opt/trn_rl_repo")
import numpy as np
import ml_dtypes

import concourse.bacc as bacc
import concourse.bass as bass
import concourse.mybir as mybir
import concourse.tile as tile
from concourse import bass_utils
from concourse.library_config import mlp as mlp_lib

NCORES = 8
P = 128
H, C = 4, 32
HC = H * C
LO = 32768          # int16 gather index limit
GCHUNK = 32         # tiles per dma_gather call
NEG_SLOPE = 0.2

_CACHE = {}
LAST_RUN_S = None   # wall time of the device dispatch (transfers + exec + fetch)


def _pad128(n):
    return (n + P - 1) // P * P


def _wrap_idx(idx_flat):
    """int16 index stream -> [128, n/16] layout (j at [j%16, j//16], x8)."""
    n = len(idx_flat)
    assert n % 16 == 0
    arr = idx_flat.reshape(-1, 16).T.astype(np.int16)
    return np.tile(arr, (8, 1)).copy()


def _host_prep(x, edge_attr, edge_index, batch, params):
    """Sort/partition edges, build per-core streams and constants."""
    N, F = x.shape
    E = edge_index.shape[1]
    assert N % NCORES == 0
    SLAB = N // NCORES
    NBLK = (SLAB + P - 1) // P
    src = edge_index[0].astype(np.int64)
    dst = edge_index[1].astype(np.int64)

    (W1, att_src1, att_dst1, We1, att_edge1,
     W2, att_src2, att_dst2, We2, att_edge2) = params

    # A matrices: [HC, H] block-diagonal per head
    def amat(att):
        A = np.zeros((HC, H), np.float32)
        for h in range(H):
            A[h * C:(h + 1) * C, h] = att[h]
        return A

    A_src1, A_dst1 = amat(att_src1), amat(att_dst1)
    A_src2, A_dst2 = amat(att_src2), amat(att_dst2)
    # a_edge[e, h] = edge_attr[e,0] * (We[0, h*C:(h+1)*C] @ att_edge[h])
    k1 = np.array([We1[0, h * C:(h + 1) * C] @ att_edge1[h] for h in range(H)], np.float32)
    k2 = np.array([We2[0, h * C:(h + 1) * C] @ att_edge2[h] for h in range(H)], np.float32)
    ea = edge_attr[:, 0].astype(np.float32)
    aedge1 = ea[:, None] * k1[None, :]
    aedge2 = ea[:, None] * k2[None, :]
    # layer-1 per-node logit pieces (host: linear in input x)
    asrc1_n = (x @ W1 @ A_src1).astype(np.float32)
    adst1_n = (x @ W1 @ A_dst1).astype(np.float32)
    s1_edge = asrc1_n[src] + adst1_n[dst] + aedge1      # [E, 4]

    core_of = dst // SLAB
    ldst = dst - core_of * SLAB
    blk = ldst // P
    dst_in_blk = ldst % P
    is_lo = src < LO

    # group edges by (core, block, half) fully vectorized
    half_f = (~is_lo).astype(np.int64)
    key = (core_of * NBLK + blk) * 2 + half_f
    order = np.argsort(key, kind="stable")
    key_s = key[order]
    ngroups = NCORES * NBLK * 2
    counts = np.bincount(key_s, minlength=ngroups)
    gstart = np.concatenate([[0], np.cumsum(counts)[:-1]])
    rank = np.arange(E, dtype=np.int64) - gstart[key_s]

    cnt3 = counts.reshape(NCORES, NBLK, 2)
    KLo = [max(1, (int(cnt3[:, b, 0].max()) + P - 1) // P) for b in range(NBLK)]
    KHi = [max(1, (int(cnt3[:, b, 1].max()) + P - 1) // P) for b in range(NBLK)]
    T_lo, T_hi = sum(KLo), sum(KHi)
    T = T_lo + T_hi
    SL, SH = T_lo * P, T_hi * P

    KLo_a, KHi_a = np.array(KLo), np.array(KHi)
    # slot base of (b, half) in processing order; lo/hi stream bases
    blk_base = np.concatenate([[0], np.cumsum(KLo_a + KHi_a)[:-1]]) * P
    lo_base = np.concatenate([[0], np.cumsum(KLo_a)[:-1]]) * P
    hi_base = np.concatenate([[0], np.cumsum(KHi_a)[:-1]]) * P
    slot_base_bh = np.stack([blk_base, blk_base + KLo_a * P], 1)  # [NBLK, 2]

    e_s = order                       # edge id per sorted position
    c_s = core_of[e_s]
    b_s = blk[e_s]
    h_s = half_f[e_s]
    slot_s = slot_base_bh[b_s, h_s] + rank
    strm_s = np.where(h_s == 0, lo_base[b_s], hi_base[b_s]) + rank

    per_core = []
    for c in range(NCORES):
        m = c_s == c
        em, sm, hm, stm = e_s[m], slot_s[m], h_s[m], strm_s[m]
        idx_lo = np.zeros(SL, np.int64)
        idx_hi = np.zeros(SH, np.int64)
        dstcol = np.full(T * P, -1.0, np.float32)      # slot -> dst_in_blk, pad -1
        dsti = np.zeros(T * P, np.int64)               # slot -> local dst (pads 0)
        s1p = np.zeros((T * P, H), np.float32)
        ae2 = np.zeros((T * P, H), np.float32)
        orig = np.full(T * P, -1, np.int64)            # slot -> original edge id
        lo_m, hi_m = hm == 0, hm == 1
        idx_lo[stm[lo_m]] = src[em[lo_m]]
        idx_hi[stm[hi_m]] = src[em[hi_m]] - LO
        dstcol[sm] = dst_in_blk[em].astype(np.float32)
        dsti[sm] = ldst[em]
        s1p[sm] = s1_edge[em]
        ae2[sm] = aedge2[em]
        orig[sm] = em
        pc = {
            "idx_lo": _wrap_idx(idx_lo.astype(np.int16)),
            "idx_hi": _wrap_idx(idx_hi.astype(np.int16)),
            "idx_dst": _wrap_idx(dsti.astype(np.int16)),
            # [128, T] / [128, 4T] layouts: slot s=(t,p) at [p, t...]
            "dstcol": dstcol.reshape(T, P).T.copy(),
            "s1p": s1p.reshape(T, P, H).transpose(1, 0, 2).reshape(P, T * H).copy(),
            "ae2": ae2.reshape(T, P, H).transpose(1, 0, 2).reshape(P, T * H).copy(),
            "orig": orig,
            "dsti": dsti,
            "x_slab": np.ascontiguousarray(x[c * SLAB:(c + 1) * SLAB]).astype(np.float32),
        }
        per_core.append(pc)

    meta = dict(N=N, F=F, E=E, SLAB=SLAB, NBLK=NBLK, KLo=KLo, KHi=KHi,
                T=T, T_lo=T_lo, T_hi=T_hi, SL=SL, SH=SH,
                A_src2=A_src2, A_dst2=A_dst2)
    return per_core, meta


def _build_module(meta, consts):
    N, F, SLAB, NBLK = meta["N"], meta["F"], meta["SLAB"], meta["NBLK"]
    KLo, KHi, T, T_lo, T_hi = meta["KLo"], meta["KHi"], meta["T"], meta["T_lo"], meta["T_hi"]
    SL, SH = meta["SL"], meta["SH"]
    SLAB_PAD = _pad128(SLAB)
    f32, bf16, i16 = mybir.dt.float32, mybir.dt.bfloat16, mybir.dt.int16
    AF = mybir.ActivationFunctionType
    OP = mybir.AluOpType

    nc = bacc.Bacc("TRN2", target_bir_lowering=False)
    # per-core inputs
    x_in = nc.dram_tensor("x_slab", [SLAB, F], f32, kind="ExternalInput")
    ilo_in = nc.dram_tensor("idx_lo", [P, SL // 16], i16, kind="ExternalInput")
    ihi_in = nc.dram_tensor("idx_hi", [P, SH // 16], i16, kind="ExternalInput")
    idst_in = nc.dram_tensor("idx_dst", [P, T * P // 16], i16, kind="ExternalInput")
    dstc_in = nc.dram_tensor("dstcol", [P, T], f32, kind="ExternalInput")
    s1p_in = nc.dram_tensor("s1p", [P, T * H], f32, kind="ExternalInput")
    ae2_in = nc.dram_tensor("ae2", [P, T * H], f32, kind="ExternalInput")
    gid_in = nc.dram_tensor("gidcol", [P, NBLK], f32, kind="ExternalInput")
    # shared consts
    cW1 = nc.dram_tensor("W1", [F, HC], f32, kind="ExternalInput")
    cW2 = nc.dram_tensor("W2", [HC, HC], f32, kind="ExternalInput")
    cA2 = nc.dram_tensor("A2", [HC, 2 * H], f32, kind="ExternalInput")
    cIottt = nc.dram_tensor("iota_row", [P, P], f32, kind="ExternalInput")
    cIotg = nc.dram_tensor("iota_g", [P, 64], f32, kind="ExternalInput")
    cIdent = nc.dram_tensor("ident", [P, P], f32, kind="ExternalInput")
    cB1 = nc.dram_tensor("b1_bcast", [P, HC], f32, kind="ExternalInput")
    cB2 = nc.dram_tensor("b2_bcast", [P, HC], f32, kind="ExternalInput")
    cW1m = nc.dram_tensor("W1m", [HC, 64], f32, kind="ExternalInput")
    cb1m = nc.dram_tensor("b1m", [1, 64], f32, kind="ExternalInput")
    cW2m = nc.dram_tensor("W2m", [64, 4], f32, kind="ExternalInput")
    cb2m = nc.dram_tensor("b2m", [1, 4], f32, kind="ExternalInput")
    cOnes = nc.dram_tensor("ones1", [1, 64], f32, kind="ExternalInput")
    # outputs
    logit_out = nc.dram_tensor("logits", [64, 4], f32, kind="ExternalOutput")
    ex1_out = nc.dram_tensor("ex1", [P, T * H], f32, kind="ExternalOutput")
    den1_out = nc.dram_tensor("den1", [P, NBLK * H], f32, kind="ExternalOutput")
    # internal DRAM
    adst2_dram = nc.dram_tensor("adst2t", [SLAB_PAD, 64], f32)

    with tile.TileContext(nc) as tc:
        with (
            tc.tile_pool(name="cst", bufs=1) as cst,
            tc.tile_pool(name="big", bufs=1) as big,
            tc.tile_pool(name="sb", bufs=3) as sb,
            tc.tile_pool(name="ring", bufs=2) as ring,
            tc.tile_pool(name="ps", bufs=1, space="PSUM") as ps,
            tc.tile_pool(name="ps1", bufs=2, space="PSUM") as ps1,
            tc.tile_pool(name="dram", bufs=1, space="DRAM") as dram,
        ):
            nc.gpsimd.load_library(mlp_lib)

            def sl2(buf, col, a=None, b=None):
                ap = buf[:, col:col + 1, :] if a is None else buf[:, col:col + 1, a:b]
                return ap.squeeze(1)

            def load_const(t_in, shape, dtype=f32, tag=None):
                t = cst.tile(shape, dtype, tag=tag or t_in.name)
                nc.sync.dma_start(t[:], t_in[:])
                return t

            W1_t = load_const(cW1, [F, HC])
            W2_t = load_const(cW2, [HC, HC])
            A2_t = load_const(cA2, [HC, 2 * H])
            iota_t = load_const(cIottt, [P, P])
            iotg_t = load_const(cIotg, [P, 64])
            id_t = load_const(cIdent, [P, P])
            b1b_t = load_const(cB1, [P, HC])
            b2b_t = load_const(cB2, [P, HC])
            W1m_t = load_const(cW1m, [HC, 64])
            b1m_t = load_const(cb1m, [1, 64])
            W2m_t = load_const(cW2m, [64, 4])
            b2m_t = load_const(cb2m, [1, 4])
            ones_t = load_const(cOnes, [1, 64])
            gid_t = load_const(gid_in, [P, NBLK])
            dstc_t = load_const(dstc_in, [P, T])
            s1p_t = big.tile([P, T * H], f32, tag="s1p")
            nc.sync.dma_start(s1p_t[:], s1p_in[:])
            ae2_t = big.tile([P, T * H], f32, tag="ae2")
            nc.sync.dma_start(ae2_t[:], ae2_in[:])
            ilo_t = big.tile([P, SL // 16], i16, tag="ilo")
            nc.sync.dma_start(ilo_t[:], ilo_in[:])
            ihi_t = big.tile([P, SH // 16], i16, tag="ihi")
            nc.sync.dma_start(ihi_t[:], ihi_in[:])
            idst_t = big.tile([P, T * P // 16], i16, tag="idst")
            nc.sync.dma_start(idst_t[:], idst_in[:])

            ex1_st = big.tile([P, T * H], f32, tag="ex1st")
            den_st = big.tile([P, NBLK * H], f32, tag="denst")
            x1_slab = big.tile([P, NBLK, HC], f32, tag="x1slab")
            x2pool = big.tile([P, NBLK, 132], bf16, tag="x2pool")
            nc.vector.memset(x2pool[:], 0.0)
            nc.vector.memset(x2pool[:, :, 128:129], 1.0)

            # collective buffers
            ag1_in = dram.tile([SLAB, HC], bf16, tag="ag1in")
            table1 = dram.tile([N, HC], bf16, tag="table1")
            ag2_in = dram.tile([SLAB, 256], bf16, tag="ag2in")
            table2 = dram.tile([N, 256], bf16, tag="table2")
            ar_in = dram.tile([64, 132], f32, tag="arin")
            ar_out = dram.tile([64, 132], f32, tag="arout")

            # ---------- phase 0: layer-1 table ----------
            for t in range(NBLK):
                rows = min(P, SLAB - t * P)
                x_t = sb.tile([P, F], f32, tag="ph0x")
                nc.sync.dma_start(x_t[:rows, :], x_in[t * P:t * P + rows, :])
                xT_p = ps.tile([P, P], f32, tag="ph0T")
                nc.tensor.transpose(xT_p[:], x_t[:], id_t[:])
                xT_s = sb.tile([P, P], f32, tag="ph0Ts")
                nc.vector.tensor_copy(xT_s[:], xT_p[:])
                h1_p = ps.tile([P, HC], f32, tag="ph0h")
                nc.tensor.matmul(h1_p[:], xT_s[:], W1_t[:], start=True, stop=True)
                h1_b = sb.tile([P, HC], bf16, tag="ph0hb")
                nc.vector.tensor_copy(h1_b[:], h1_p[:])
                nc.sync.dma_start(ag1_in[t * P:t * P + rows, :], h1_b[:rows, :])
            nc.gpsimd.collective_compute(
                "AllGather", OP.bypass, replica_groups=[list(range(NCORES))],
                ins=[ag1_in.opt()], outs=[table1.opt()])

            # ---------- edge phase (shared for both layers) ----------
            def edge_phase(layer):
                tbl = table1 if layer == 1 else table2
                elem = HC if layer == 1 else 256
                # gather chunk schedules for lo/hi streams
                chunks = {}
                for half, Ttot, it, base in (
                    ("lo", T_lo, ilo_t, 0), ("hi", T_hi, ihi_t, LO)):
                    lst = []
                    for c0 in range(0, Ttot, GCHUNK):
                        nt = min(GCHUNK, Ttot - c0)
                        lst.append((c0, nt))
                    chunks[half] = lst
                g_bufs = {}

                def get_g(half, st_tile):
                    # returns (buf, col) for stream-tile index
                    cidx = st_tile // GCHUNK
                    c0 = cidx * GCHUNK
                    nt = min(GCHUNK, (T_lo if half == "lo" else T_hi) - c0)
                    key = (half, cidx)
                    if key not in g_bufs:
                        it = ilo_t if half == "lo" else ihi_t
                        src_ap = tbl[:, :] if (half == "lo" or N <= LO) else tbl[LO:, :]
                        g = ring.tile([P, GCHUNK, elem], bf16, tag=f"g{layer}")
                        nc.gpsimd.dma_gather(
                            g[:, :nt, :], src_ap, it[:, c0 * 8:(c0 + nt) * 8],
                            nt * P, nt * P, elem, single_packet=False)
                        gd = None
                        g_bufs[key] = g
                    return g_bufs[key], st_tile - c0

                gd_bufs = {}

                def get_gd(gt):
                    cidx = gt // GCHUNK
                    c0 = cidx * GCHUNK
                    nt = min(GCHUNK, T - c0)
                    if cidx not in gd_bufs:
                        g = ring.tile([P, GCHUNK, 64], f32, tag="gd")
                        nc.gpsimd.dma_gather(
                            g[:, :nt, :], adst2_dram[:, :],
                            idst_t[:, c0 * 8:(c0 + nt) * 8],
                            nt * P, nt * P, 64, single_packet=False)
                        gd_bufs[cidx] = g
                    return gd_bufs[cidx], gt - c0

                lo_t, hi_t, gt = 0, 0, 0
                for b in range(NBLK):
                    out_p = ps1.tile([P, 132], f32, tag="out")
                    ntile = KLo[b] + KHi[b]
                    done = 0
                    for half, K in (("lo", KLo[b]), ("hi", KHi[b])):
                        for k in range(K):
                            st = lo_t if half == "lo" else hi_t
                            gbuf, gcol = get_g(half, st)
                            # s logits [P, 4]
                            if layer == 1:
                                s_ap = s1p_t[:, gt * H:(gt + 1) * H]
                            else:
                                gdbuf, gdcol = get_gd(gt)
                                t0 = sb.tile([P, H], f32, tag="t0")
                                nc.vector.tensor_tensor(
                                    t0[:], sl2(gbuf, gcol, 128, 132),
                                    ae2_t[:, gt * H:(gt + 1) * H], OP.add)
                                s_t2 = sb.tile([P, H], f32, tag="s2")
                                nc.vector.tensor_tensor(
                                    s_t2[:], t0[:], sl2(gdbuf, gdcol, 0, H), OP.add)
                                s_ap = s_t2[:]
                            lk1 = sb.tile([P, H], f32, tag="lk1")
                            nc.vector.tensor_scalar_mul(lk1[:], s_ap, NEG_SLOPE)
                            lk = sb.tile([P, H], f32, tag="lk")
                            nc.vector.tensor_tensor(lk[:], lk1[:], s_ap, OP.max)
                            msgx = sb.tile([P, 132], bf16, tag="msgx")
                            nc.scalar.activation(msgx[:, 128:132], lk[:], AF.Exp)
                            if layer == 1:
                                nc.scalar.activation(
                                    ex1_st[:, gt * H:(gt + 1) * H], lk[:], AF.Exp)
                            exv = msgx[:, 128:132].unsqueeze(2).to_broadcast([P, H, C])
                            nc.vector.tensor_tensor(
                                msgx[:, :128].rearrange("p (h c) -> p h c", h=H),
                                sl2(gbuf, gcol, 0, 128).rearrange("p (h c) -> p h c", h=H),
                                exv, OP.mult)
                            S_t = sb.tile([P, P], bf16, tag="S")
                            nc.vector.tensor_scalar(
                                S_t[:], iota_t[:], dstc_t[:, gt:gt + 1], None, OP.is_equal)
                            nc.tensor.matmul(out_p[:], S_t[:], msgx[:],
                                             start=(done == 0), stop=(done == ntile - 1))
                            done += 1
                            gt += 1
                            if half == "lo":
                                lo_t += 1
                            else:
                                hi_t += 1
                    # block epilogue
                    if layer == 1:
                        nc.vector.tensor_copy(den_st[:, b * H:(b + 1) * H], out_p[:, 128:132])
                    dpe = sb.tile([P, H], f32, tag="dpe")
                    nc.vector.tensor_scalar_add(dpe[:], out_p[:, 128:132], 1e-16)
                    drec = sb.tile([P, H], f32, tag="drec")
                    nc.vector.reciprocal(drec[:], dpe[:])
                    xo = sb.tile([P, HC], f32, tag="xo")
                    nc.vector.tensor_tensor(
                        xo[:].rearrange("p (h c) -> p h c", h=H),
                        out_p[:, :128].rearrange("p (h c) -> p h c", h=H),
                        drec[:].unsqueeze(2).to_broadcast([P, H, C]), OP.mult)
                    bias_t = b1b_t if layer == 1 else b2b_t
                    nc.vector.tensor_tensor(xo[:], xo[:], bias_t[:], OP.add)
                    # elu(xo) = max(xo,0) + exp(min(xo,0)) - 1
                    mneg = sb.tile([P, HC], f32, tag="mneg")
                    nc.vector.tensor_scalar_min(mneg[:], xo[:], 0.0)
                    epos = sb.tile([P, HC], f32, tag="epos")
                    nc.scalar.activation(epos[:], mneg[:], AF.Exp)
                    rpos = sb.tile([P, HC], f32, tag="rpos")
                    nc.vector.tensor_scalar_max(rpos[:], xo[:], 0.0)
                    xn = sb.tile([P, HC], f32, tag="xn")
                    nc.vector.tensor_tensor(xn[:], epos[:], rpos[:], OP.add)
                    if layer == 1:
                        nc.vector.tensor_scalar_add(sl2(x1_slab, b), xn[:], -1.0)
                    else:
                        nc.vector.tensor_scalar_add(sl2(x2pool, b, 0, 128), xn[:], -1.0)

            edge_phase(1)

            # ---------- phase 0': layer-2 table ----------
            adst_st = sb.tile([P, NBLK, H], f32, tag="adstst")
            for t in range(NBLK):
                rows = min(P, SLAB - t * P)
                xT_p = ps.tile([P, P], f32, tag="ph0T")
                nc.tensor.transpose(xT_p[:], sl2(x1_slab, t), id_t[:])
                xT_s = sb.tile([P, P], f32, tag="ph0Ts")
                nc.vector.tensor_copy(xT_s[:], xT_p[:])
                h2_p = ps.tile([P, HC], f32, tag="ph0h")
                nc.tensor.matmul(h2_p[:], xT_s[:], W2_t[:], start=True, stop=True)
                h2T_p = ps.tile([P, P], f32, tag="ph0hT")
                nc.tensor.matmul(h2T_p[:], W2_t[:], xT_s[:], start=True, stop=True)
                h2T_s = sb.tile([P, P], f32, tag="ph0hTs")
                nc.vector.tensor_copy(h2T_s[:], h2T_p[:])
                aa_p = ps.tile([P, 2 * H], f32, tag="ph0aa")
                nc.tensor.matmul(aa_p[:], h2T_s[:], A2_t[:], start=True, stop=True)
                row_t = sb.tile([P, 256], bf16, tag="ph0row")
                nc.vector.memset(row_t[:], 0.0)
                nc.vector.tensor_copy(row_t[:, :128], h2_p[:])
                nc.vector.tensor_copy(row_t[:, 128:132], aa_p[:, 0:H])
                nc.vector.tensor_copy(sl2(adst_st, t), aa_p[:, H:2 * H])
                nc.sync.dma_start(ag2_in[t * P:t * P + rows, :], row_t[:rows, :])
            adst_view = adst2_dram[:].rearrange("(t p) c -> p t c", p=P)
            nc.sync.dma_start(adst_view[:, :, 0:H], adst_st[:])
            nc.gpsimd.collective_compute(
                "AllGather", OP.bypass, replica_groups=[list(range(NCORES))],
                ins=[ag2_in.opt()], outs=[table2.opt()])

            edge_phase(2)

            # ---------- pooling ----------
            pool_p = ps1.tile([64, 132], f32, tag="pool")
            for b in range(NBLK):
                Sg = sb.tile([P, 64], bf16, tag="Sg")
                nc.vector.tensor_scalar(
                    Sg[:], iotg_t[:], gid_t[:, b:b + 1], None, OP.is_equal)
                nc.tensor.matmul(pool_p[:], Sg[:], sl2(x2pool, b),
                                 start=(b == 0), stop=(b == NBLK - 1))
            pool_s = sb.tile([64, 132], f32, tag="pools")
            nc.vector.tensor_copy(pool_s[:], pool_p[:])
            nc.gpsimd.dma_start(ar_in[:], pool_s[:])
            nc.gpsimd.collective_compute(
                "AllReduce", OP.add, replica_groups=[list(range(NCORES))],
                ins=[ar_in.opt()], outs=[ar_out.opt()])
            arr = sb.tile([64, 132], f32, tag="arr")
            nc.sync.dma_start(arr[:], ar_out[:])

            # ---------- MLP ----------
            cnt = sb.tile([64, 1], f32, tag="cnt")
            nc.vector.tensor_scalar_max(cnt[:], arr[:, 128:129], 1.0)
            crec = sb.tile([64, 1], f32, tag="crec")
            nc.vector.reciprocal(crec[:], cnt[:])
            gmat = sb.tile([64, 128], f32, tag="gmat")
            nc.vector.tensor_scalar(gmat[:], arr[:, :128], crec[:], None, OP.mult)
            gT_p = ps.tile([P, 64], f32, tag="ph0h")
            nc.tensor.transpose(gT_p[:], gmat[:], id_t[:64, :64])
            gT_s = sb.tile([P, 64], f32, tag="gTs")
            nc.vector.tensor_copy(gT_s[:], gT_p[:])
            m1_p = ps.tile([64, 64], f32, tag="ph0aa")
            nc.tensor.matmul(m1_p[:], gT_s[:], W1m_t[:], start=True, stop=False)
            nc.tensor.matmul(m1_p[:], ones_t[:], b1m_t[:], start=False, stop=True)
            hrelu = sb.tile([64, 64], f32, tag="hrelu")
            nc.vector.tensor_scalar_max(hrelu[:], m1_p[:], 0.0)
            hT_p = ps.tile([64, 64], f32, tag="ph0hT")
            nc.tensor.transpose(hT_p[:], hrelu[:], id_t[:64, :64])
            hT_s = sb.tile([64, 64], f32, tag="hTs")
            nc.vector.tensor_copy(hT_s[:], hT_p[:])
            m2_p = ps.tile([64, 4], f32, tag="ph0T")
            nc.tensor.matmul(m2_p[:], hT_s[:], W2m_t[:], start=True, stop=False)
            nc.tensor.matmul(m2_p[:], ones_t[:, :64], b2m_t[:], start=False, stop=True)
            lg = sb.tile([64, 4], f32, tag="lg")
            nc.vector.tensor_copy(lg[:], m2_p[:])
            nc.sync.dma_start(logit_out[:], lg[:])

            # stream outputs
            nc.sync.dma_start(ex1_out[:], ex1_st[:])
            nc.sync.dma_start(den1_out[:], den_st[:])
    nc.compile()
    return nc


def kernel(x, edge_attr, W1, att_src1, att_dst1, We1, att_edge1, b1,
           W2, att_src2, att_dst2, We2, att_edge2, b2,
           mlp_W1, mlp_b1, bn_gamma, bn_beta, bn_mean, bn_var, mlp_W2, mlp_b2,
           edge_index, batch):
    x = np.asarray(x, np.float32)
    edge_attr = np.asarray(edge_attr, np.float32)
    edge_index_in = edge_index
    edge_index = np.asarray(edge_index)
    batch = np.asarray(batch)
    params = tuple(np.asarray(p, np.float32) for p in (
        W1, att_src1, att_dst1, We1, att_edge1,
        W2, att_src2, att_dst2, We2, att_edge2))
    b1 = np.asarray(b1, np.float32)
    b2 = np.asarray(b2, np.float32)

    per_core, meta = _host_prep(x, edge_attr, edge_index, batch, params)
    N, E, SLAB, NBLK, T = meta["N"], meta["E"], meta["SLAB"], meta["NBLK"], meta["T"]

    # consts
    ident = np.eye(P, dtype=np.float32)
    iota_row = np.broadcast_to(np.arange(P, dtype=np.float32)[None, :], (P, P)).copy()
    iota_g = np.broadcast_to(np.arange(64, dtype=np.float32)[None, :], (P, 64)).copy()
    b1_bcast = np.broadcast_to(b1[None, :], (P, HC)).copy()
    b2_bcast = np.broadcast_to(b2[None, :], (P, HC)).copy()
    A2cat = np.concatenate([meta["A_src2"], meta["A_dst2"]], 1).astype(np.float32)
    # fold BN (eval) into mlp layer 1
    scale = (np.asarray(bn_gamma, np.float32)
             / np.sqrt(np.asarray(bn_var, np.float32) + 1e-5))
    shift = np.asarray(bn_beta, np.float32) - np.asarray(bn_mean, np.float32) * scale
    W1m = (np.asarray(mlp_W1, np.float32) * scale[None, :]).astype(np.float32)
    b1m = (np.asarray(mlp_b1, np.float32) * scale + shift).astype(np.float32)[None, :]
    W2m = np.asarray(mlp_W2, np.float32)
    b2m = np.asarray(mlp_b2, np.float32)[None, :]
    ones1 = np.ones((1, 64), np.float32)
    gid = np.asarray(batch, np.int64)

    shared = {
        "W1": params[0], "W2": params[5], "A2": A2cat,
        "iota_row": iota_row, "iota_g": iota_g, "ident": ident,
        "b1_bcast": b1_bcast, "b2_bcast": b2_bcast,
        "W1m": W1m, "b1m": b1m, "W2m": W2m, "b2m": b2m, "ones1": ones1,
    }
    key = (N, E, tuple(meta["KLo"]), tuple(meta["KHi"]))
    if key not in _CACHE:
        _CACHE[key] = _build_module(meta, shared)
    nc = _CACHE[key]

    in_maps = []
    for c in range(NCORES):
        pc = per_core[c]
        gidc = np.full((NBLK * P,), -1.0, np.float32)
        gslab = gid[c * SLAB:(c + 1) * SLAB].astype(np.float32)
        gidc[:SLAB] = gslab
        m = dict(shared)
        m.update({
            "x_slab": pc["x_slab"], "idx_lo": pc["idx_lo"], "idx_hi": pc["idx_hi"],
            "idx_dst": pc["idx_dst"], "dstcol": pc["dstcol"],
            "s1p": pc["s1p"], "ae2": pc["ae2"],
            "gidcol": gidc.reshape(NBLK, P).T.copy(),
        })
        in_maps.append(m)

    import time as _time
    _t0 = _time.perf_counter()
    res = bass_utils.run_bass_kernel_spmd(nc, in_maps, core_ids=list(range(NCORES)))
    outs = res.results
    global LAST_RUN_S
    LAST_RUN_S = _time.perf_counter() - _t0

    logits = outs[0]["logits"].astype(np.float32)

    # alpha1 on host: ex / (denom[dst] + 1e-16)
    alpha = np.zeros((E, H), np.float32)
    for c in range(NCORES):
        pc = per_core[c]
        ex = outs[c]["ex1"].reshape(P, T, H).transpose(1, 0, 2).reshape(T * P, H)
        den = outs[c]["den1"].reshape(P, NBLK, H).transpose(1, 0, 2).reshape(NBLK * P, H)
        real = pc["orig"] >= 0
        d_lidx = pc["dsti"][real]
        blkpos = (d_lidx // P) * P + (d_lidx % P)
        alpha[pc["orig"][real]] = ex[real] / (den[blkpos] + 1e-16)

    return logits, (edge_index_in, alpha)
